# revision 11
# baseline (speedup 1.0000x reference)
"""GQA attention kernel for Trainium2, 8-core tensor-parallel (by heads).

Shapes (hardcoded from the problem spec):
  x:(4,128,4096) fp32, wq:(4096,4096), wk/wv:(4096,1024), wo:(4096,4096),
  32 q heads / 8 kv heads, head_dim 128, start_pos=0 (cache is overwritten).

Sharding: core c owns q heads [4c,4c+4) and kv head c; wq/wk/wv column-
sharded, wo row-sharded; each core computes a full (512,4096) partial of
the output projection; host sums the 8 partials and adds bo.

v2: all streamed tensors (x, wq, wkv, wo) are cast to bf16 on the host and
DMA'd as bf16 (half the HBM traffic of v1; matmul rate is unchanged at
1 cyc/row). DRAM layouts are rearranged chunk-major so each DMA moves
>=512KB. Attention internals (scores, softmax weights, v) run in bf16;
PSUM accumulation and the output partials stay fp32.
"""
import sys
sys.path.insert(0, "/opt/trn_rl_repo")

import numpy as np

B, S, D = 4, 128, 4096
H, KV, HD = 32, 8, 128
NCORES = 8
HQ = H // NCORES          # 4 q heads per core
T = B * S                 # 512 tokens
FQ = HQ * HD              # 512 q features per core
SCALE = 1.0 / float(np.sqrt(HD))
NK = D // 128             # 32 contraction chunks
SUB = 4                   # chunks per DMA block
NB = NK // SUB            # 8 DMA blocks

_CACHE = {}


def _build(reps=1):
    import concourse.bass as bass
    import concourse.tile as tile
    from concourse import bacc, mybir

    F32 = mybir.dt.float32
    BF = mybir.dt.bfloat16
    AF = mybir.ActivationFunctionType

    nc = bacc.Bacc("TRN2", target_bir_lowering=False, debug=False,
                   enable_asserts=False, num_devices=NCORES)

    # chunk-major layouts: column block k holds contraction chunk k
    xT_d = nc.dram_tensor("xT", [128, NK * T], BF, kind="ExternalInput").ap()
    wq_d = nc.dram_tensor("wq", [128, NK * FQ], BF, kind="ExternalInput").ap()
    wkv_d = nc.dram_tensor("wkv", [128, NK * 2 * HD], BF, kind="ExternalInput").ap()
    wo_d = nc.dram_tensor("wo", [128, (D // 512) * HQ * 512], BF,
                          kind="ExternalInput").ap()
    bq_d = nc.dram_tensor("bq", [1, FQ], F32, kind="ExternalInput").ap()
    bkv_d = nc.dram_tensor("bkv", [1, 2 * HD], F32, kind="ExternalInput").ap()
    c4_d = nc.dram_tensor("c4", [S, HQ * 64], F32, kind="ExternalInput").ap()
    s4_d = nc.dram_tensor("s4", [S, HQ * 64], F32, kind="ExternalInput").ap()
    mk_d = nc.dram_tensor("mk", [S, HQ * S], BF, kind="ExternalInput").ap()
    on_d = nc.dram_tensor("on", [S, S], BF, kind="ExternalInput").ap()
    id_d = nc.dram_tensor("idm", [S, S], BF, kind="ExternalInput").ap()
    out_d = nc.dram_tensor("out", [T, D], F32, kind="ExternalOutput").ap()

    with tile.TileContext(nc) as tc, \
         nc.allow_low_precision(reason="bf16 transpose passthrough, no accumulation"):
        with tc.tile_pool(name="consts", bufs=1) as cp:
            b128q = cp.tile([128, FQ], F32)
            b128kv = cp.tile([128, 2 * HD], F32)
            c4 = cp.tile([128, HQ * 64], F32)
            s4 = cp.tile([128, HQ * 64], F32)
            mk = cp.tile([128, HQ * S], BF)
            ones = cp.tile([128, S], BF)
            ident = cp.tile([128, S], BF)
            nc.gpsimd.dma_start(b128q, bass.AP(tensor=bq_d.tensor, offset=0,
                                               ap=[[0, 128], bq_d.ap[1]]))
            nc.gpsimd.dma_start(b128kv, bass.AP(tensor=bkv_d.tensor, offset=0,
                                                ap=[[0, 128], bkv_d.ap[1]]))
            nc.sync.dma_start(c4, c4_d)
            nc.sync.dma_start(s4, s4_d)
            nc.sync.dma_start(mk, mk_d)
            nc.sync.dma_start(ones, on_d)
            nc.sync.dma_start(ident, id_d)

            with tc.tile_pool(name="qkvs", bufs=4) as qp, \
                 tc.tile_pool(name="ropep", bufs=4) as rp, \
                 tc.tile_pool(name="tmpp", bufs=2) as tp, \
                 tc.tile_pool(name="trs", bufs=4) as trp, \
                 tc.tile_pool(name="attn", bufs=2) as ap_, \
                 tc.tile_pool(name="aop", bufs=4) as aop, \
                 tc.tile_pool(name="outp", bufs=8) as op:
              for rep in range(reps):
                R = f"r{rep}_"
                # ---------------- Phase A: QKV projections -------------
                q_sb = [None] * B
                kv_sb = [None] * B
                with tc.tile_pool(name=R + "psA", bufs=1, space="PSUM") as psA, \
                     tc.tile_pool(name=R + "xtp", bufs=3) as xp, \
                     tc.tile_pool(name=R + "wp", bufs=3) as wp:
                    pq = [psA.tile([128, FQ], F32, tag=f"pq{m}", name=f"{R}pq{m}")
                          for m in range(B)]
                    pkv = [psA.tile([128, 2 * HD], F32, tag=f"pkv{m}",
                                    name=f"{R}pkv{m}") for m in range(B)]
                    for kk in range(NB):
                        xt = xp.tile([128, SUB * T], BF, tag="xt", name=f"{R}xt{kk}")
                        (nc.sync if kk % 2 == 0 else nc.scalar).dma_start(
                            xt, xT_d[:, kk * SUB * T:(kk + 1) * SUB * T])
                        wqt = wp.tile([128, SUB * FQ], BF, tag="wqt", name=f"{R}wqt{kk}")
                        (nc.scalar if kk % 2 == 0 else nc.sync).dma_start(
                            wqt, wq_d[:, kk * SUB * FQ:(kk + 1) * SUB * FQ])
                        wkvt = wp.tile([128, SUB * 2 * HD], BF, tag="wkvt",
                                       name=f"{R}wkvt{kk}")
                        nc.scalar.dma_start(
                            wkvt, wkv_d[:, kk * SUB * 2 * HD:(kk + 1) * SUB * 2 * HD])
                        for sub in range(SUB):
                            first = (kk == 0 and sub == 0)
                            last = (kk == NB - 1 and sub == SUB - 1)
                            for m in range(B):
                                lhs = xt[:, sub * T + m * 128:sub * T + (m + 1) * 128]
                                nc.tensor.matmul(
                                    pq[m], lhs,
                                    wqt[:, sub * FQ:(sub + 1) * FQ],
                                    start=first, stop=last)
                                nc.tensor.matmul(
                                    pkv[m], lhs,
                                    wkvt[:, sub * 2 * HD:(sub + 1) * 2 * HD],
                                    start=first, stop=last)
                    for m in range(B):
                        q_sb[m] = qp.tile([128, FQ], F32, tag="q", name=f"{R}q{m}")
                        nc.vector.tensor_add(q_sb[m], pq[m], b128q)
                        kv_sb[m] = qp.tile([128, 2 * HD], F32, tag="kv", name=f"{R}kv{m}")
                        nc.vector.tensor_add(kv_sb[m], pkv[m], b128kv)

                # ------------- Phases B-D per batch tile ---------------
                with tc.tile_pool(name=R + "psB", bufs=1, space="PSUM") as psB, \
                     tc.tile_pool(name=R + "wop", bufs=1) as wop:
                    NT = D // 512  # 8 output-column tiles
                    wots = []
                    for n in range(NT):
                        wot = wop.tile([128, HQ * 512], BF, tag=f"wo{n}",
                                       name=f"{R}wo{n}")
                        (nc.scalar if n % 2 == 0 else nc.sync).dma_start(
                            wot, wo_d[:, n * HQ * 512:(n + 1) * HQ * 512])
                        wots.append(wot)
                    c4v = c4.rearrange("p (h r) -> p h r", h=HQ)
                    s4v = s4.rearrange("p (h r) -> p h r", h=HQ)
                    aoT = [None] * B
                    for m in range(B):
                        # RoPE on q (fp32 math, bf16 result)
                        qv = q_sb[m].rearrange("p (h r two) -> p h r two",
                                               h=HQ, r=64, two=2)
                        q_e, q_o = qv[:, :, :, 0], qv[:, :, :, 1]
                        qr = rp.tile([128, FQ], BF, tag="qr", name=f"{R}qr{m}")
                        qrv = qr.rearrange("p (h r two) -> p h r two",
                                           h=HQ, r=64, two=2)
                        t1 = tp.tile([128, HQ * 64], F32, tag="t1", name=f"{R}t1_{m}")
                        t2 = tp.tile([128, HQ * 64], F32, tag="t2", name=f"{R}t2_{m}")
                        t1v = t1.rearrange("p (h r) -> p h r", h=HQ)
                        t2v = t2.rearrange("p (h r) -> p h r", h=HQ)
                        nc.vector.tensor_mul(t1v, q_o, s4v)
                        nc.vector.tensor_mul(t2v, q_e, c4v)
                        nc.vector.tensor_sub(qrv[:, :, :, 0], t2v, t1v)
                        nc.vector.tensor_mul(t1v, q_o, c4v)
                        nc.vector.tensor_mul(t2v, q_e, s4v)
                        nc.vector.tensor_add(qrv[:, :, :, 1], t2v, t1v)
                        # RoPE on k (head 0 of kv tile)
                        kv_ = kv_sb[m][:, 0:HD].rearrange("p (r two) -> p r two",
                                                          r=64, two=2)
                        k_e, k_o = kv_[:, :, 0], kv_[:, :, 1]
                        kr = rp.tile([128, HD], BF, tag="kr", name=f"{R}kr{m}")
                        krv = kr.rearrange("p (r two) -> p r two", r=64, two=2)
                        t1k = t1v[:, 0, :]
                        t2k = t2v[:, 0, :]
                        c1 = c4v[:, 0, :]
                        s1 = s4v[:, 0, :]
                        nc.vector.tensor_mul(t1k, k_o, s1)
                        nc.vector.tensor_mul(t2k, k_e, c1)
                        nc.vector.tensor_sub(krv[:, :, 0], t2k, t1k)
                        nc.vector.tensor_mul(t1k, k_o, c1)
                        nc.vector.tensor_mul(t2k, k_e, s1)
                        nc.vector.tensor_add(krv[:, :, 1], t2k, t1k)
                        # v cast to bf16 for the PV matmul
                        v_bf = rp.tile([128, HD], BF, tag="vbf", name=f"{R}vbf{m}")
                        nc.vector.tensor_copy(v_bf, kv_sb[m][:, HD:2 * HD])

                        # Transposes -> one psum bank: q heads at 0..511, k at 512..639
                        pst = psB.tile([128, FQ + HD], BF, tag="pst", bufs=2,
                                       name=f"{R}pst{m}")
                        for h in range(HQ):
                            nc.tensor.transpose(pst[:, h * 128:(h + 1) * 128],
                                                qr[:, h * 128:(h + 1) * 128], ident)
                        nc.tensor.transpose(pst[:, FQ:FQ + HD], kr, ident)
                        qT = trp.tile([128, FQ], BF, tag="qT", name=f"{R}qT{m}")
                        nc.vector.tensor_copy(qT, pst[:, 0:FQ])
                        kT = trp.tile([128, HD], BF, tag="kT", name=f"{R}kT{m}")
                        nc.scalar.copy(kT, pst[:, FQ:FQ + HD])

                        # Attention (scoresT layout [j,(h,i)]); PV runs on the
                        # unnormalized weights, normalization applied after.
                        psc = psB.tile([128, FQ], F32, tag="psc", name=f"{R}psc{m}")
                        nc.tensor.matmul(psc, kT, qT, start=True, stop=True)
                        expT = ap_.tile([128, FQ], BF, tag="expT", name=f"{R}expT{m}")
                        nc.scalar.activation(expT, psc, AF.Exp, scale=SCALE)
                        attn_u = ap_.tile([128, FQ], BF, tag="attn_u", name=f"{R}au{m}")
                        nc.vector.tensor_mul(attn_u, expT, mk)
                        pden = psB.tile([128, FQ], F32, tag="pden", name=f"{R}pden{m}")
                        nc.tensor.matmul(pden, ones, attn_u, start=True, stop=True)
                        rec = ap_.tile([128, FQ], F32, tag="rec", name=f"{R}rec{m}")
                        nc.vector.reciprocal(rec, pden)
                        poT = psB.tile([128, FQ], F32, tag="poT", bufs=2,
                                       name=f"{R}poT{m}")
                        nc.tensor.matmul(poT, v_bf, attn_u, start=True, stop=True)
                        aoT[m] = aop.tile([128, FQ], BF, tag="aoT", name=f"{R}aoT{m}")
                        nc.vector.tensor_mul(aoT[m], poT, rec)

                        # ---- Phase D for this batch: output projection ----
                        for n in range(NT):
                            pso = psB.tile([128, 512], F32, tag="pso", bufs=2,
                                           name=f"{R}pso{n}_{m}")
                            for h in range(HQ):
                                nc.tensor.matmul(pso, aoT[m][:, h * 128:(h + 1) * 128],
                                                 wots[n][:, h * 512:(h + 1) * 512],
                                                 start=(h == 0),
                                                 stop=(h == HQ - 1))
                            osb = op.tile([128, 512], F32, tag="osb",
                                          name=f"{R}osb{n}_{m}")
                            if (n * B + m) % 2 == 0:
                                nc.vector.tensor_copy(osb, pso)
                            else:
                                nc.scalar.copy(osb, pso)
                            nc.sync.dma_start(
                                out_d[m * 128:(m + 1) * 128,
                                      n * 512:(n + 1) * 512], osb)

    nc.compile()
    return nc


def _to_bf16(a):
    import ml_dtypes
    return np.ascontiguousarray(a.astype(ml_dtypes.bfloat16))


def _chunk_major(a2d):
    """[D, C] -> [128, NK*C]: column block k holds rows k*128:(k+1)*128."""
    d, c = a2d.shape
    nk = d // 128
    return np.ascontiguousarray(
        a2d.reshape(nk, 128, c).transpose(1, 0, 2).reshape(128, nk * c))


def _prep_inputs(x, freqs_cos, freqs_sin, wq, bq, wk, bk, wv, bv, wo):
    import ml_dtypes
    xT = np.ascontiguousarray(x.reshape(T, D).T.astype(np.float32))
    xTr = _to_bf16(_chunk_major(xT))
    c4 = np.ascontiguousarray(np.tile(freqs_cos.astype(np.float32), (1, HQ)))
    s4 = np.ascontiguousarray(np.tile(freqs_sin.astype(np.float32), (1, HQ)))
    mk = _to_bf16(np.tile(np.triu(np.ones((S, S), np.float32)), (1, HQ)))
    on = _to_bf16(np.ones((S, S), np.float32))
    idm = _to_bf16(np.eye(S, dtype=np.float32))
    maps = []
    for c in range(NCORES):
        qs = slice(c * FQ, (c + 1) * FQ)
        ks = slice(c * HD, (c + 1) * HD)
        wo_loc = wo[qs, :].astype(np.float32)         # [FQ, D]
        # [128, NT*HQ*512]: per output-column tile n, the HQ row-chunks
        wo_r = wo_loc.reshape(HQ, 128, D // 512, 512).transpose(
            1, 2, 0, 3).reshape(128, -1)
        maps.append({
            "xT": xTr,
            "wq": _to_bf16(_chunk_major(wq[:, qs].astype(np.float32))),
            "wkv": _to_bf16(_chunk_major(np.concatenate(
                [wk[:, ks], wv[:, ks]], axis=1).astype(np.float32))),
            "wo": _to_bf16(wo_r),
            "bq": np.ascontiguousarray(bq[qs].astype(np.float32)).reshape(1, FQ),
            "bkv": np.ascontiguousarray(
                np.concatenate([bk[ks], bv[ks]]).astype(np.float32)).reshape(1, 2 * HD),
            "c4": c4, "s4": s4, "mk": mk, "on": on, "idm": idm,
        })
    return maps


def kernel(x, start_pos, freqs_cos, freqs_sin, mask, cache_k, cache_v,
           wq, bq, wk, bk, wv, bv, wo, bo, _want_trace=False):
    from concourse.bass_utils import run_bass_kernel_spmd

    assert int(start_pos) == 0
    if "nc" not in _CACHE:
        _CACHE["nc"] = _build()
    nc = _CACHE["nc"]
    in_maps = _prep_inputs(np.asarray(x), np.asarray(freqs_cos),
                           np.asarray(freqs_sin), np.asarray(wq),
                           np.asarray(bq), np.asarray(wk), np.asarray(bk),
                           np.asarray(wv), np.asarray(bv), np.asarray(wo))
    res = run_bass_kernel_spmd(nc, in_maps, core_ids=list(range(NCORES)),
                               trace=_want_trace)
    acc = np.zeros((T, D), np.float64)
    for r in res.results:
        acc += r["out"].astype(np.float64)
    out = (acc + np.asarray(bo).astype(np.float64)).astype(np.float32)
    if _want_trace:
        _CACHE["last_exec_time_ns"] = res.exec_time_ns
        _CACHE["last_trace"] = res.instructions_and_trace
    return out.reshape(B, S, D)


# revision 13
# speedup vs baseline: 1.1058x; 1.1058x over previous
"""GQA attention kernel for Trainium2, 8-core tensor-parallel (by heads).

Shapes (hardcoded from the problem spec):
  x:(4,128,4096) fp32, wq:(4096,4096), wk/wv:(4096,1024), wo:(4096,4096),
  32 q heads / 8 kv heads, head_dim 128, start_pos=0 (cache is overwritten).

Sharding: core c owns q heads [4c,4c+4) and kv head c; wq/wk/wv column-
sharded, wo row-sharded; each core computes a full (512,4096) partial of
the output projection; host sums the 8 partials and adds bo.

v2: all streamed tensors (x, wq, wkv, wo) are cast to bf16 on the host and
DMA'd as bf16 (half the HBM traffic of v1; matmul rate is unchanged at
1 cyc/row). DRAM layouts are rearranged chunk-major so each DMA moves
>=512KB. Attention internals (scores, softmax weights, v) run in bf16;
PSUM accumulation and the output partials stay fp32.
"""
import sys
sys.path.insert(0, "/opt/trn_rl_repo")

import numpy as np

B, S, D = 4, 128, 4096
H, KV, HD = 32, 8, 128
NCORES = 8
HQ = H // NCORES          # 4 q heads per core
T = B * S                 # 512 tokens
FQ = HQ * HD              # 512 q features per core
SCALE = 1.0 / float(np.sqrt(HD))
NK = D // 128             # 32 contraction chunks
SUB = 4                   # chunks per DMA block
NB = NK // SUB            # 8 DMA blocks

_CACHE = {}


def _build(reps=1):
    import concourse.bass as bass
    import concourse.tile as tile
    from concourse import bacc, mybir

    F32 = mybir.dt.float32
    BF = mybir.dt.bfloat16
    AF = mybir.ActivationFunctionType

    nc = bacc.Bacc("TRN2", target_bir_lowering=False, debug=False,
                   enable_asserts=False, num_devices=NCORES)

    # chunk-major layouts: column block k holds contraction chunk k
    xT_d = nc.dram_tensor("xT", [128, NK * T], BF, kind="ExternalInput").ap()
    wq_d = nc.dram_tensor("wq", [128, NK * FQ], BF, kind="ExternalInput").ap()
    wkv_d = nc.dram_tensor("wkv", [128, NK * 2 * HD], BF, kind="ExternalInput").ap()
    wo_d = nc.dram_tensor("wo", [128, (D // 512) * HQ * 512], BF,
                          kind="ExternalInput").ap()
    bq_d = nc.dram_tensor("bq", [1, FQ], F32, kind="ExternalInput").ap()
    bkv_d = nc.dram_tensor("bkv", [1, 2 * HD], F32, kind="ExternalInput").ap()
    c4_d = nc.dram_tensor("c4", [S, HQ * 64], F32, kind="ExternalInput").ap()
    s4_d = nc.dram_tensor("s4", [S, HQ * 64], F32, kind="ExternalInput").ap()
    mk_d = nc.dram_tensor("mk", [S, HQ * S], BF, kind="ExternalInput").ap()
    on_d = nc.dram_tensor("on", [S, S], BF, kind="ExternalInput").ap()
    id_d = nc.dram_tensor("idm", [S, S], BF, kind="ExternalInput").ap()
    out_d = nc.dram_tensor("out", [T, D], F32, kind="ExternalOutput").ap()

    with tile.TileContext(nc) as tc, \
         nc.allow_low_precision(reason="bf16 transpose passthrough, no accumulation"):
        with tc.tile_pool(name="consts", bufs=1) as cp:
            b128q = cp.tile([128, FQ], F32)
            b128kv = cp.tile([128, 2 * HD], F32)
            c4 = cp.tile([128, HQ * 64], F32)
            s4 = cp.tile([128, HQ * 64], F32)
            mk = cp.tile([128, HQ * S], BF)
            ones = cp.tile([128, S], BF)
            ident = cp.tile([128, S], BF)
            nc.gpsimd.dma_start(b128q, bass.AP(tensor=bq_d.tensor, offset=0,
                                               ap=[[0, 128], bq_d.ap[1]]))
            nc.gpsimd.dma_start(b128kv, bass.AP(tensor=bkv_d.tensor, offset=0,
                                                ap=[[0, 128], bkv_d.ap[1]]))
            nc.sync.dma_start(c4, c4_d)
            nc.sync.dma_start(s4, s4_d)
            nc.sync.dma_start(mk, mk_d)
            nc.sync.dma_start(ones, on_d)
            nc.sync.dma_start(ident, id_d)

            with tc.tile_pool(name="qkvs", bufs=4) as qp, \
                 tc.tile_pool(name="ropep", bufs=4) as rp, \
                 tc.tile_pool(name="tmpp", bufs=2) as tp, \
                 tc.tile_pool(name="trs", bufs=4) as trp, \
                 tc.tile_pool(name="attn", bufs=2) as ap_, \
                 tc.tile_pool(name="aop", bufs=4) as aop, \
                 tc.tile_pool(name="outp", bufs=8) as op:
              for rep in range(reps):
                R = f"r{rep}_"
                # ---------------- Phase A: QKV projections -------------
                q_sb = [None] * B
                kv_sb = [None] * B
                with tc.tile_pool(name=R + "psA", bufs=1, space="PSUM") as psA, \
                     tc.tile_pool(name=R + "xtp", bufs=4) as xp, \
                     tc.tile_pool(name=R + "wp", bufs=4) as wp:
                    pq = [psA.tile([128, FQ], F32, tag=f"pq{m}", name=f"{R}pq{m}")
                          for m in range(B)]
                    pkv = [psA.tile([128, 2 * HD], F32, tag=f"pkv{m}",
                                    name=f"{R}pkv{m}") for m in range(B)]
                    for kk in range(NB):
                        xt = xp.tile([128, SUB * T], BF, tag="xt", name=f"{R}xt{kk}")
                        (nc.sync if kk % 2 == 0 else nc.scalar).dma_start(
                            xt, xT_d[:, kk * SUB * T:(kk + 1) * SUB * T])
                        wqt = wp.tile([128, SUB * FQ], BF, tag="wqt", name=f"{R}wqt{kk}")
                        (nc.scalar if kk % 2 == 0 else nc.sync).dma_start(
                            wqt, wq_d[:, kk * SUB * FQ:(kk + 1) * SUB * FQ])
                        wkvt = wp.tile([128, SUB * 2 * HD], BF, tag="wkvt",
                                       name=f"{R}wkvt{kk}")
                        nc.scalar.dma_start(
                            wkvt, wkv_d[:, kk * SUB * 2 * HD:(kk + 1) * SUB * 2 * HD])
                        for sub in range(SUB):
                            first = (kk == 0 and sub == 0)
                            last = (kk == NB - 1 and sub == SUB - 1)
                            for m in range(B):
                                lhs = xt[:, sub * T + m * 128:sub * T + (m + 1) * 128]
                                nc.tensor.matmul(
                                    pq[m], lhs,
                                    wqt[:, sub * FQ:(sub + 1) * FQ],
                                    start=first, stop=last)
                                nc.tensor.matmul(
                                    pkv[m], lhs,
                                    wkvt[:, sub * 2 * HD:(sub + 1) * 2 * HD],
                                    start=first, stop=last)
                    for m in range(B):
                        q_sb[m] = qp.tile([128, FQ], F32, tag="q", name=f"{R}q{m}")
                        nc.vector.tensor_add(q_sb[m], pq[m], b128q)
                        kv_sb[m] = qp.tile([128, 2 * HD], F32, tag="kv", name=f"{R}kv{m}")
                        nc.vector.tensor_add(kv_sb[m], pkv[m], b128kv)

                # ------------- Phases B-D per batch tile ---------------
                with tc.tile_pool(name=R + "psB", bufs=1, space="PSUM") as psB, \
                     tc.tile_pool(name=R + "wop", bufs=1) as wop:
                    NT = D // 512  # 8 output-column tiles
                    wots = []
                    for n in range(NT):
                        wot = wop.tile([128, HQ * 512], BF, tag=f"wo{n}",
                                       name=f"{R}wo{n}")
                        nc.gpsimd.dma_start(
                            wot, wo_d[:, n * HQ * 512:(n + 1) * HQ * 512])
                        wots.append(wot)
                    c4v = c4.rearrange("p (h r) -> p h r", h=HQ)
                    s4v = s4.rearrange("p (h r) -> p h r", h=HQ)
                    aoT = [None] * B
                    for m in range(B):
                        # RoPE on q (fp32 math, bf16 result)
                        qv = q_sb[m].rearrange("p (h r two) -> p h r two",
                                               h=HQ, r=64, two=2)
                        q_e, q_o = qv[:, :, :, 0], qv[:, :, :, 1]
                        qr = rp.tile([128, FQ], BF, tag="qr", name=f"{R}qr{m}")
                        qrv = qr.rearrange("p (h r two) -> p h r two",
                                           h=HQ, r=64, two=2)
                        t1 = tp.tile([128, HQ * 64], F32, tag="t1", name=f"{R}t1_{m}")
                        t2 = tp.tile([128, HQ * 64], F32, tag="t2", name=f"{R}t2_{m}")
                        t1v = t1.rearrange("p (h r) -> p h r", h=HQ)
                        t2v = t2.rearrange("p (h r) -> p h r", h=HQ)
                        nc.vector.tensor_mul(t1v, q_o, s4v)
                        nc.vector.tensor_mul(t2v, q_e, c4v)
                        nc.vector.tensor_sub(qrv[:, :, :, 0], t2v, t1v)
                        nc.vector.tensor_mul(t1v, q_o, c4v)
                        nc.vector.tensor_mul(t2v, q_e, s4v)
                        nc.vector.tensor_add(qrv[:, :, :, 1], t2v, t1v)
                        # RoPE on k (head 0 of kv tile)
                        kv_ = kv_sb[m][:, 0:HD].rearrange("p (r two) -> p r two",
                                                          r=64, two=2)
                        k_e, k_o = kv_[:, :, 0], kv_[:, :, 1]
                        kr = rp.tile([128, HD], BF, tag="kr", name=f"{R}kr{m}")
                        krv = kr.rearrange("p (r two) -> p r two", r=64, two=2)
                        t1k = t1v[:, 0, :]
                        t2k = t2v[:, 0, :]
                        c1 = c4v[:, 0, :]
                        s1 = s4v[:, 0, :]
                        nc.vector.tensor_mul(t1k, k_o, s1)
                        nc.vector.tensor_mul(t2k, k_e, c1)
                        nc.vector.tensor_sub(krv[:, :, 0], t2k, t1k)
                        nc.vector.tensor_mul(t1k, k_o, c1)
                        nc.vector.tensor_mul(t2k, k_e, s1)
                        nc.vector.tensor_add(krv[:, :, 1], t2k, t1k)
                        # v cast to bf16 for the PV matmul
                        v_bf = rp.tile([128, HD], BF, tag="vbf", name=f"{R}vbf{m}")
                        nc.vector.tensor_copy(v_bf, kv_sb[m][:, HD:2 * HD])

                        # Transposes -> one psum bank: q heads at 0..511, k at 512..639
                        pst = psB.tile([128, FQ + HD], BF, tag="pst", bufs=2,
                                       name=f"{R}pst{m}")
                        for h in range(HQ):
                            nc.tensor.transpose(pst[:, h * 128:(h + 1) * 128],
                                                qr[:, h * 128:(h + 1) * 128], ident)
                        nc.tensor.transpose(pst[:, FQ:FQ + HD], kr, ident)
                        qT = trp.tile([128, FQ], BF, tag="qT", name=f"{R}qT{m}")
                        nc.vector.tensor_copy(qT, pst[:, 0:FQ])
                        kT = trp.tile([128, HD], BF, tag="kT", name=f"{R}kT{m}")
                        nc.scalar.copy(kT, pst[:, FQ:FQ + HD])

                        # Attention (scoresT layout [j,(h,i)]); PV runs on the
                        # unnormalized weights, normalization applied after.
                        psc = psB.tile([128, FQ], F32, tag="psc", name=f"{R}psc{m}")
                        nc.tensor.matmul(psc, kT, qT, start=True, stop=True)
                        expT = ap_.tile([128, FQ], BF, tag="expT", name=f"{R}expT{m}")
                        nc.scalar.activation(expT, psc, AF.Exp, scale=SCALE)
                        attn_u = ap_.tile([128, FQ], BF, tag="attn_u", name=f"{R}au{m}")
                        nc.vector.tensor_mul(attn_u, expT, mk)
                        pden = psB.tile([128, FQ], F32, tag="pden", name=f"{R}pden{m}")
                        nc.tensor.matmul(pden, ones, attn_u, start=True, stop=True)
                        rec = ap_.tile([128, FQ], F32, tag="rec", name=f"{R}rec{m}")
                        nc.vector.reciprocal(rec, pden)
                        poT = psB.tile([128, FQ], F32, tag="poT", bufs=2,
                                       name=f"{R}poT{m}")
                        nc.tensor.matmul(poT, v_bf, attn_u, start=True, stop=True)
                        aoT[m] = aop.tile([128, FQ], BF, tag="aoT", name=f"{R}aoT{m}")
                        nc.vector.tensor_mul(aoT[m], poT, rec)

                        # ---- Phase D for this batch: output projection ----
                        for n in range(NT):
                            pso = psB.tile([128, 512], F32, tag="pso", bufs=2,
                                           name=f"{R}pso{n}_{m}")
                            for h in range(HQ):
                                nc.tensor.matmul(pso, aoT[m][:, h * 128:(h + 1) * 128],
                                                 wots[n][:, h * 512:(h + 1) * 512],
                                                 start=(h == 0),
                                                 stop=(h == HQ - 1))
                            osb = op.tile([128, 512], F32, tag="osb",
                                          name=f"{R}osb{n}_{m}")
                            if (n * B + m) % 2 == 0:
                                nc.vector.tensor_copy(osb, pso)
                            else:
                                nc.scalar.copy(osb, pso)
                            nc.sync.dma_start(
                                out_d[m * 128:(m + 1) * 128,
                                      n * 512:(n + 1) * 512], osb)

    nc.compile()
    return nc


def _to_bf16(a):
    import ml_dtypes
    return np.ascontiguousarray(a.astype(ml_dtypes.bfloat16))


def _chunk_major(a2d):
    """[D, C] -> [128, NK*C]: column block k holds rows k*128:(k+1)*128."""
    d, c = a2d.shape
    nk = d // 128
    return np.ascontiguousarray(
        a2d.reshape(nk, 128, c).transpose(1, 0, 2).reshape(128, nk * c))


def _prep_inputs(x, freqs_cos, freqs_sin, wq, bq, wk, bk, wv, bv, wo):
    import ml_dtypes
    xT = np.ascontiguousarray(x.reshape(T, D).T.astype(np.float32))
    xTr = _to_bf16(_chunk_major(xT))
    c4 = np.ascontiguousarray(np.tile(freqs_cos.astype(np.float32), (1, HQ)))
    s4 = np.ascontiguousarray(np.tile(freqs_sin.astype(np.float32), (1, HQ)))
    mk = _to_bf16(np.tile(np.triu(np.ones((S, S), np.float32)), (1, HQ)))
    on = _to_bf16(np.ones((S, S), np.float32))
    idm = _to_bf16(np.eye(S, dtype=np.float32))
    maps = []
    for c in range(NCORES):
        qs = slice(c * FQ, (c + 1) * FQ)
        ks = slice(c * HD, (c + 1) * HD)
        wo_loc = wo[qs, :].astype(np.float32)         # [FQ, D]
        # [128, NT*HQ*512]: per output-column tile n, the HQ row-chunks
        wo_r = wo_loc.reshape(HQ, 128, D // 512, 512).transpose(
            1, 2, 0, 3).reshape(128, -1)
        maps.append({
            "xT": xTr,
            "wq": _to_bf16(_chunk_major(wq[:, qs].astype(np.float32))),
            "wkv": _to_bf16(_chunk_major(np.concatenate(
                [wk[:, ks], wv[:, ks]], axis=1).astype(np.float32))),
            "wo": _to_bf16(wo_r),
            "bq": np.ascontiguousarray(bq[qs].astype(np.float32)).reshape(1, FQ),
            "bkv": np.ascontiguousarray(
                np.concatenate([bk[ks], bv[ks]]).astype(np.float32)).reshape(1, 2 * HD),
            "c4": c4, "s4": s4, "mk": mk, "on": on, "idm": idm,
        })
    return maps


def kernel(x, start_pos, freqs_cos, freqs_sin, mask, cache_k, cache_v,
           wq, bq, wk, bk, wv, bv, wo, bo, _want_trace=False):
    from concourse.bass_utils import run_bass_kernel_spmd

    assert int(start_pos) == 0
    if "nc" not in _CACHE:
        _CACHE["nc"] = _build()
    nc = _CACHE["nc"]
    in_maps = _prep_inputs(np.asarray(x), np.asarray(freqs_cos),
                           np.asarray(freqs_sin), np.asarray(wq),
                           np.asarray(bq), np.asarray(wk), np.asarray(bk),
                           np.asarray(wv), np.asarray(bv), np.asarray(wo))
    res = run_bass_kernel_spmd(nc, in_maps, core_ids=list(range(NCORES)),
                               trace=_want_trace)
    acc = np.zeros((T, D), np.float64)
    for r in res.results:
        acc += r["out"].astype(np.float64)
    out = (acc + np.asarray(bo).astype(np.float64)).astype(np.float32)
    if _want_trace:
        _CACHE["last_exec_time_ns"] = res.exec_time_ns
        _CACHE["last_trace"] = res.instructions_and_trace
    return out.reshape(B, S, D)
